# revision 25
# baseline (speedup 1.0000x reference)
"""Trainium2 Bass kernel for nn_AttentionModel (sparse_attention greedy decode).

B=128, N=1000, E=128, H=8, d=16. 999 sequential greedy-decode steps.
Data-parallel over batch: 16 batch elements per core x 8 cores.

Per-core design (all heavy state SBUF/PSUM-resident, bf16 matmuls):
  - neT/kT/lkT (e-major) + V (n-major) built on device from one GEMM pass.
  - scores PSUM [128p=n%128, (c,b,h)=1024] is a *persistent accumulator*:
    per step it gains K.(q_t - q_{t-1}) via 256 small matmuls (q telescopes)
    plus a rank-16 one-hot outer-product matmul that adds -50 at the newly
    visited node (attention mask; exp(-50)~2e-22). Softmax numerator weights
    come from a single Exp activation over the psum.
  - ctx per head via V-chunk matmuls; per-head normalization via a
    ones-matmul denominator + reciprocal + K=1 broadcast matmul.
  - logits PSUM [128p, (b,c)] accumulates lk.(x_t - x_{t-1}); mask -1e9 is a
    separate persistent psum added after tanh (tanh computed as
    1 - 2/(exp(2z)+1) so only Exp/Ln/Copy ACT tables are used -> no table
    set thrashing).
  - argmax: pack round(tanh*4096)*8 + chunk_idx into fp32, reduce-max over
    chunks, PE-transpose, DVE max/max_index -> per-batch node index + value.
  - log-sum-exp via ones-matmul over exp(10*tanh) + Ln.
  - next-step embedding fetched with one indirect DMA gather from HBM.
"""
import sys
import json

sys.path.insert(0, '/opt/trn_rl_repo')

import numpy as np
import concourse.bass as bass
import concourse.mybir as mybir
import concourse.tile as tile
from concourse.bass_utils import run_bass_kernel_spmd

dt = mybir.dt
AF = mybir.ActivationFunctionType
ALU = mybir.AluOpType

# ---------------------------------------------------------------- dims
B, N, E, H, D = 128, 1000, 128, 8, 16
NCORES = 8
BC = B // NCORES          # 16 batches per core
C = 8                     # node chunks of 128
NP = C * 128              # 1024 padded nodes
STEPS = N - 1             # 999
SCALE = 1.0 / np.sqrt(D)  # 0.25
PACKS = 4096.0            # argmax value quantization
RBIG = 12582912.0         # 1.5 * 2^23 fp32 round-to-int trick
MASK_ATT = -50.0
MASK_LOG = -1e9
TANH_CLIP = 10.0

LN_FUNC = getattr(AF, "Ln", None) or getattr(AF, "Log")
STATIC_MAX = 4  # steps <= this get statically unrolled (no For_i)


# ------------------------------------------------- walrus wait-limit fix
def _split_waits(bir_bytes: bytes, max_waits: int = 1) -> bytes:
    """This walrus build rejects >1 sync wait per instruction. Move overflow
    waits onto NoOp carriers inserted before the offender (same engine)."""
    d = json.loads(bir_bytes)
    ctr = [0]
    for fn in d["functions"]:
        for blk in fn["blocks"]:
            out = []
            for inst in blk["instructions"]:
                si = inst.get("sync_info") or {}
                waits = si.get("on_wait") or []
                if len(waits) > max_waits:
                    overflow = waits[:-max_waits]
                    for i in range(0, len(overflow), max_waits):
                        ctr[0] += 1
                        out.append({
                            "debug": inst.get("debug", 0),
                            "engine": inst["engine"],
                            "ins": [], "outs": [],
                            "name": f"waitfix-{ctr[0]}",
                            "opcode": "NoOp",
                            "sync_info": {"on_update": [],
                                          "on_wait": overflow[i:i + max_waits]},
                        })
                    si = dict(si)
                    si["on_wait"] = waits[-max_waits:]
                    inst = dict(inst)
                    inst["sync_info"] = si
                out.append(inst)
            blk["instructions"] = out
    return json.dumps(d).encode()


def _swdge_to_evsem(inst):
    """Rewrite InstIncSwdgeSem (unencodable by this walrus build) into an
    EventSemaphore doing the same semaphore arithmetic (For_i loop-invariance
    bookkeeping: add/sub on DMASW sems at the skip/reset blocks)."""
    if inst.get("op_name") != "InstIncSwdgeSem" or inst.get("opcode") != "ISA":
        return inst
    upd_mode = "sem-inc" if inst.get("mode", "add") == "add" else "sem-dec"
    base = inst["sem_id_base"]
    si = inst.get("sync_info") or {}
    out = []
    first = True
    for i, v in enumerate(inst.get("sem_values") or []):
        for k in range(int(v)):
            out.append({"debug": inst.get("debug", 0), "engine": inst["engine"],
                        "ins": [], "outs": [],
                        "name": f"{inst['name']}-sw{i}-{k}",
                        "opcode": "EventSemaphore",
                        "sync_info": {
                            "on_update": [{"ant_name": inst["sem_names"][i],
                                           "id": base + i,
                                           "sync_type": "semaphore",
                                           "update_mode": upd_mode,
                                           "update_value": 1}],
                            "on_wait": ((si.get("on_wait") or []) if first else [])
                            or [{"ant_name": inst["sem_names"][i],
                                 "id": base + i, "sync_type": "semaphore",
                                 "wait_mode": "sem-ge-imm", "wait_value": 0}]}})
            first = False
    if not out:
        out = [{"debug": inst.get("debug", 0), "engine": inst["engine"],
                "ins": [], "outs": [], "name": inst["name"], "opcode": "NoOp",
                "sync_info": {"on_update": [], "on_wait": si.get("on_wait") or []}}]
    # preserve any on_update the original carried (unlikely) on the last carrier
    extra_upd = si.get("on_update") or []
    if extra_upd:
        out[-1]["sync_info"]["on_update"] = (
            out[-1]["sync_info"]["on_update"] + extra_upd)
    return out


_orig_split_body = _split_waits


def _fix_bir(bir_bytes: bytes) -> bytes:
    d = json.loads(bir_bytes)
    ctr = [0]
    for fn in d["functions"]:
        for blk in fn["blocks"]:
            out = []
            expanded = []
            for inst in blk["instructions"]:
                r = _swdge_to_evsem(inst)
                if isinstance(r, list):
                    expanded.extend(r)
                else:
                    expanded.append(r)
            for inst in expanded:
                si = inst.get("sync_info") or {}
                waits = si.get("on_wait") or []
                if len(waits) > 1:
                    for w in waits[:-1]:
                        ctr[0] += 1
                        out.append({"debug": inst.get("debug", 0),
                                    "engine": inst["engine"],
                                    "ins": [], "outs": [],
                                    "name": f"waitfix-{ctr[0]}",
                                    "opcode": "NoOp",
                                    "sync_info": {"on_update": [],
                                                  "on_wait": [w]}})
                    si = dict(si)
                    si["on_wait"] = waits[-1:]
                    inst = dict(inst)
                    inst["sync_info"] = si
                out.append(inst)
            blk["instructions"] = out
    return json.dumps(d).encode()


if not getattr(bass.Bass, "_waitfix_patched", False):
    _orig_to_json = bass.Bass.to_json_bytes
    bass.Bass.to_json_bytes = lambda self: _fix_bir(_orig_to_json(self))
    bass.Bass._waitfix_patched = True


# ---------------------------------------------------------------- build
def build_kernel(steps: int = STEPS, debug: bool = False) -> bass.Bass:
    nc = bass.Bass()
    f32, bf16, u32 = dt.float32, dt.bfloat16, dt.uint32

    P = lambda name, shape: nc.declare_dram_parameter(name, shape, f32, isOutput=False)
    ne = P("ne", [BC * N, E])           # flat node embeddings (gather table)
    geT = P("geT", [E, BC])
    Wk = P("Wk", [E, E]); Wv = P("Wv", [E, E]); Wlk = P("Wlk", [E, E])
    bkT = P("bkT", [E, 1]); blkT = P("blkT", [E, 1]); bvR = P("bvR", [1, E])
    Wfx = P("Wfx", [E, E]); W1 = P("W1", [E, E]); Wq2 = P("Wq2", [E, E])
    bq1R = P("bq1R", [1, E])
    Wml = P("Wml", [E, E])
    ctxp0 = P("ctxp0", [E, BC])
    ident = P("ident", [128, 128])
    ohBIG = P("ohBIG", [128, 128])      # [e,(b,h)] 1 iff e//16==h
    CIDX = P("CIDX", [128, 128])        # [p,(b,c)] value c
    CBIDX = P("CBIDX", [BC, NP])        # [b,(c,b',h)] c if b'==b else -1
    CBIDX2 = P("CBIDX2", [BC, 128])     # [b,(b',c)] c if b'==b else -1
    iotaP = P("iotaP", [BC, 128])       # [b,p] = p
    padU = P("padU", [1, 128])          # 1 at p>=104
    padW2 = P("padW2", [1, NP])         # MASK_ATT at cols c==7
    padW3 = P("padW3", [1, 128])        # MASK_LOG at cols (b,c==7)
    U0 = P("U0", [BC, 128])             # -50 onehot(p=0)
    U01 = P("U01", [BC, 128])           # 1.0 onehot(p=0)
    U09 = P("U09", [BC, 128])           # -1e9 onehot(p=0)
    W20 = P("W20", [BC, NP])            # node-0 att mask rhs
    W30 = P("W30", [BC, 128])           # node-0 logit mask rhs

    out_total = nc.declare_dram_parameter("total", [BC, 1], f32, isOutput=True)
    dbg = {}
    if debug:
        def DBG(name, shape, dty=f32):
            dbg[name] = nc.declare_dram_parameter(name, shape, dty, isOutput=True)
        DBG("d_ptil", [128, NP]); DBG("d_lm", [128, 128]); DBG("d_th", [128, 128])
        DBG("d_mx", [BC, 8]); DBG("d_mi", [BC, 8], u32); DBG("d_nst", [BC, 1])
        DBG("d_ctxT", [E, BC]); DBG("d_sumw", [1, BC]); DBG("d_lse", [BC, 1])
        DBG("d_kT", [128, 1024]); DBG("d_v", [128, 1024]); DBG("d_qb", [E, BC])
        DBG("d_scores", [128, NP]); DBG("d_logits", [128, 128])

    from contextlib import ExitStack
    with tile.TileContext(nc) as tc, ExitStack() as es:
        sb = es.enter_context(tc.tile_pool(name="sb", bufs=1))
        ps = es.enter_context(tc.tile_pool(name="ps", bufs=1, space="PSUM"))

        pre = tc.tile_pool(name="pre", bufs=1)
        prep = pre.__enter__()

        # ---------------- load + cast weights/constants
        _params = {"geT": geT, "Wk": Wk, "Wv": Wv, "Wlk": Wlk,
                   "bkT": bkT, "blkT": blkT, "bvR": bvR,
                   "Wfx": Wfx, "W1": W1, "Wq2": Wq2,
                   "bq1R": bq1R, "Wml": Wml, "ctxp0": ctxp0,
                   "ident": ident, "ohBIG": ohBIG,
                   "CIDX": CIDX, "CBIDX": CBIDX,
                   "CBIDX2": CBIDX2, "iotaP": iotaP,
                   "padU": padU, "padW2": padW2,
                   "padW3": padW3, "U0": U0, "U09": U09,
                   "W20": W20, "W30": W30, "U01": U01}

        def load(pname, shape, cast_bf=False, persist=False):
            """Stage a param; staging f32 goes in the prologue pool, the
            returned tile lives in `sb` iff persist."""
            stage_pool = sb if (persist and not cast_bf) else prep
            tag = "ldstage" if cast_bf else f"ld_{pname}"
            t = stage_pool.tile(shape, f32, tag=tag)
            nc.sync.dma_start(t[:], _params[pname][:])
            if not cast_bf:
                return t
            tb = (sb if persist else prep).tile(shape, bf16, tag=f"ldb_{pname}")
            nc.vector.tensor_copy(tb[:], t[:])
            return tb

        geT_b = load("geT", [E, BC], True)
        Wk_b = load("Wk", [E, E], True)
        Wv_b = load("Wv", [E, E], True)
        Wlk_b = load("Wlk", [E, E], True)
        bkT_s = load("bkT", [E, 1])
        blkT_s = load("blkT", [E, 1])
        bvR_b = load("bvR", [1, E], True)
        Wfx_b = load("Wfx", [E, E], True)
        W1_b = load("W1", [E, E], True)
        Wq2_s = load("Wq2", [E, E], persist=True)   # fp32 for the small q matmul
        bq1R_b = load("bq1R", [1, E], True)
        Wml_b = load("Wml", [E, E], True, persist=True)
        ctxp0_s = load("ctxp0", [E, BC])
        ident_s = load("ident", [128, 128], persist=True)
        ohBIG_s = load("ohBIG", [128, 128], persist=True)
        CIDX_s = load("CIDX", [128, 128], persist=True)
        CBIDX_s = load("CBIDX", [BC, NP], persist=True)
        CBIDX2_s = load("CBIDX2", [BC, 128], persist=True)
        iotaP_s = load("iotaP", [BC, 128], persist=True)
        padU_b = load("padU", [1, 128], True)
        padW2_b = load("padW2", [1, NP], True)
        padW3_b = load("padW3", [1, 128], True)
        U0_s = load("U0", [BC, 128])
        U01_s = load("U01", [BC, 128])
        U09_s = load("U09", [BC, 128])
        W20_s = load("W20", [BC, NP])
        W30_s = load("W30", [BC, 128])

        onesCb = sb.tile([128, 1], bf16)       # ones column (Z matmuls lhsT)
        nc.vector.memset(onesCb[:], 1.0)
        onesRb = sb.tile([1, 128], bf16)       # ones row
        nc.vector.memset(onesRb[:], 1.0)
        zeroRb = prep.tile([1, 128], bf16)
        nc.vector.memset(zeroRb[:], 0.0)

        # ---------------- big SBUF tensors
        neTb = prep.tile([128, BC * N], bf16)  # [e, b*1000+n] (prologue only)
        kTall = sb.tile([128, BC * NP], bf16)  # [e, (b*8+c)*128+p']
        lkTall = sb.tile([128, BC * NP], bf16)
        Vall = sb.tile([128, BC * NP], bf16)   # [n%128, (b*8+c)*128+e]
        NEall = sb.tile([128, BC * NP], bf16)  # raw ne, same layout as Vall
        Ptil = sb.tile([128, NP], bf16)        # softmax weights [p,(c,b,h)]
        nc.vector.memset(kTall[:], 0.0)
        nc.vector.memset(lkTall[:], 0.0)
        nc.vector.memset(Vall[:], 0.0)
        nc.vector.memset(NEall[:], 0.0)

        # ---------------- persistent PSUM accumulators
        scoresP = ps.tile([128, NP], f32)      # [p,(c,b,h)] : K.q_t + masks
        lgP = ps.tile([128, 256], f32)         # [:, :128]=logits (b,c), [:,128:]=maskBIG
        logitsP = lgP[:, 0:128]
        maskBIGp = lgP[:, 128:256]
        cxP = ps.tile([128, 256], f32)         # [:, :128]=ctxH (b,h), [:,128:]=rZb
        ctxHp = cxP[:, 0:128]
        rZbp = cxP[:, 128:256]
        smP = ps.tile([128, 512], f32)         # shared transients
        lastTp = smP[:, 0:16]
        dqTp = smP[:, 16:32]
        dxTp = smP[:, 32:48]
        Zp = smP[0:1, 64:192]
        Z2p = smP[0:1, 192:320]
        lseTp = smP[0:16, 320:321]
        trP = ps.tile([128, 128], f32)         # transpose scratch (pcT / neT build)

        # ---------------- loop state tiles
        mU = sb.tile([BC, 128], bf16)
        mU1 = sb.tile([BC, 128], bf16)
        mU9 = sb.tile([BC, 128], bf16)
        mW2 = sb.tile([BC, NP], bf16)
        mW3 = sb.tile([BC, 128], bf16)
        lastTprev = sb.tile([E, BC], f32)
        ctxTprev = sb.tile([E, BC], f32)
        total = sb.tile([BC, 1], f32)
        Qd = sb.tile([128, 128], bf16)         # [e,(b,h)] q-delta blockdiag

        nc.vector.tensor_copy(mU[:], U0_s[:])
        nc.vector.tensor_copy(mU1[:], U01_s[:])
        nc.vector.tensor_copy(mU9[:], U09_s[:])
        nc.vector.tensor_copy(mW2[:], W20_s[:])
        nc.vector.tensor_copy(mW3[:], W30_s[:])
        nc.vector.memset(lastTprev[:], 0.0)
        nc.vector.tensor_copy(ctxTprev[:], ctxp0_s[:])
        nc.vector.memset(total[:], 0.0)

        # ---------------- build neT (transpose ne into e-major, bf16)
        TCH = 5  # tiles per load round
        with tc.tile_pool(name="nef", bufs=1) as nefp:
            ne_r = ne.rearrange("(t p) e -> p t e", p=128)  # t = 125
            for L in range(125 // TCH):
                neF = nefp.tile([128, TCH * 128], f32, tag="neF")
                nc.sync.dma_start(
                    neF[:].rearrange("p (t e) -> p t e", e=128),
                    ne_r[:, L * TCH:(L + 1) * TCH, :])
                for j in range(TCH):
                    t = L * TCH + j
                    nc.tensor.transpose(trP[:], neF[:, j * 128:(j + 1) * 128],
                                        ident_s[:])
                    nc.vector.tensor_copy(neTb[:, t * 128:(t + 1) * 128], trP[:])

        # ---------------- kT / lkT projections (+bias), layout [e,(b,c,p')]
        kps = ps.tile([128, 512], f32)
        for (Wb, bT, dst) in ((Wk_b, bkT_s, kTall), (Wlk_b, blkT_s, lkTall)):
            for b in range(BC):
                for (o, w) in ((0, 512), (512, 488)):
                    nc.tensor.matmul(kps[:, :w], lhsT=Wb[:],
                                     rhs=neTb[:, b * N + o: b * N + o + w],
                                     start=True, stop=True)
                    nc.vector.tensor_scalar(
                        out=dst[:, b * NP + o: b * NP + o + w],
                        in0=kps[:, :w], scalar1=bT[:, 0:1], scalar2=None,
                        op0=ALU.add)

        # ---------------- V (+bias), layout [n%128, (b,c,e)]
        for b in range(BC):
            for c in range(C):
                ncols = 128 if c < 7 else N - 7 * 128  # 104 for last chunk
                nc.tensor.matmul(trP[:], lhsT=onesRb[:], rhs=bvR_b[:],
                                 start=True, stop=False)
                nc.tensor.matmul(
                    trP[:ncols, :],
                    lhsT=neTb[:, b * N + c * 128: b * N + c * 128 + ncols],
                    rhs=Wv_b[:], start=False, stop=True)
                nc.vector.tensor_copy(Vall[:, (b * 8 + c) * 128:(b * 8 + c + 1) * 128],
                                      trP[:])

        # ---------------- NEall: raw embeddings in [n%128, (b,c,e)] layout
        nesc = prep.tile([128, NP], f32, tag="nesc")
        for b in range(BC):
            nc.vector.memset(nesc[:], 0.0)
            nc.sync.dma_start(
                nesc[:].rearrange("p (c e) -> p c e", e=E)[:, 0:7, :],
                ne[b * N: b * N + 896, :].rearrange("(c p) e -> p c e", p=128))
            nc.sync.dma_start(
                nesc[0:104, 7 * E:8 * E],
                ne[b * N + 896: b * N + 1000, :])
            nc.vector.tensor_copy(NEall[:, b * NP:(b + 1) * NP], nesc[:])

        # ---------------- qbase^T and initial scores = K.qbase
        qbps = smP[:, 48:64]
        nc.tensor.matmul(qbps, lhsT=Wfx_b[:], rhs=geT_b[:], start=True, stop=False)
        firstT = neTb[:].rearrange("e (b n) -> e b n", n=N)[:, :, 0:1]
        nc.tensor.matmul(qbps, lhsT=W1_b[:],
                         rhs=firstT, start=False, stop=False)
        nc.tensor.matmul(qbps, lhsT=bq1R_b[:], rhs=onesRb[:, :BC],
                         start=False, stop=True)
        if debug:
            qbs = sb.tile([E, BC], f32)
            nc.vector.tensor_copy(qbs[:], qbps)
            nc.sync.dma_start(dbg["d_qb"][:], qbs[:])
        nc.vector.tensor_tensor(
            out=Qd[:].rearrange("p (b h) -> p b h", h=H),
            in0=qbps.rearrange("p (b o) -> p b o", o=1).to_broadcast([128, BC, H]),
            in1=ohBIG_s[:].rearrange("p (b h) -> p b h", h=H),
            op=ALU.mult)
        for b in range(BC):
            for c in range(C):
                nc.tensor.matmul(
                    scoresP[:, c * 128 + b * 8: c * 128 + b * 8 + 8],
                    lhsT=kTall[:, (b * 8 + c) * 128:(b * 8 + c + 1) * 128],
                    rhs=Qd[:, b * 8:(b + 1) * 8], start=True, stop=True)
        nc.tensor.matmul(scoresP[:, 0:512], lhsT=padU_b[:], rhs=padW2_b[:, 0:512],
                         start=False, stop=True)
        nc.tensor.matmul(scoresP[:, 512:1024], lhsT=padU_b[:],
                         rhs=padW2_b[:, 512:1024], start=False, stop=True)
        # maskBIG init (pads) / logits zero-init
        nc.tensor.matmul(maskBIGp, lhsT=padU_b[:], rhs=padW3_b[:],
                         start=True, stop=True)
        nc.tensor.matmul(logitsP, lhsT=zeroRb[:], rhs=padW3_b[:],
                         start=True, stop=True)

        pre.__exit__(None, None, None)  # free prologue staging SBUF

        # ---------------- working tiles for the loop
        U8s = sb.tile([128, 128], bf16)
        dlT = sb.tile([E, BC], f32)
        cxs = sb.tile([128, 128], f32)
        t5 = sb.tile([128, 128], f32)
        t6 = sb.tile([128, 128], f32)
        ctxT = sb.tile([E, BC], f32)
        dcT = sb.tile([E, BC], bf16)
        dxT = sb.tile([E, BC], bf16)
        rZ = sb.tile([1, 128], bf16)
        rZf = sb.tile([1, 128], f32)
        e2 = sb.tile([128, 128], f32)
        ep1 = sb.tile([128, 128], f32)
        rp = sb.tile([128, 128], f32)
        th = sb.tile([128, 128], f32)
        lm = sb.tile([128, 128], f32)
        w2 = sb.tile([128, 128], bf16)
        pk1 = sb.tile([128, 128], f32)
        packed = sb.tile([128, 128], f32)
        pc = sb.tile([128, BC], f32)
        pcTs = sb.tile([BC, 128], f32)
        mx = sb.tile([BC, 8], f32)
        mi = sb.tile([BC, 8], u32)
        sumw = sb.tile([1, BC], f32)
        lse1 = sb.tile([1, BC], f32)
        lseTs = sb.tile([BC, 1], f32)
        pf = sb.tile([BC, 1], f32)
        d1 = sb.tile([BC, 1], f32)
        qv = sb.tile([BC, 1], f32)
        tt1 = sb.tile([BC, 1], f32)
        cst = sb.tile([BC, 1], f32)
        tt2 = sb.tile([BC, 1], f32)
        nst = sb.tile([BC, 1], f32)
        mv10 = sb.tile([BC, 1], f32)
        lp = sb.tile([BC, 1], f32)

        def body():
            # mask outer-products for the previously chosen node
            nc.tensor.matmul(scoresP[:, 0:512], lhsT=mU[:], rhs=mW2[:, 0:512],
                             start=False, stop=True)
            nc.tensor.matmul(scoresP[:, 512:1024], lhsT=mU[:],
                             rhs=mW2[:, 512:1024], start=False, stop=True)
            nc.tensor.matmul(maskBIGp, lhsT=mU9[:], rhs=mW3[:],
                             start=False, stop=True)

            # gather current-node embeddings via one-hot matmuls; q delta
            nc.tensor.matmul(trP[:], lhsT=mU1[:], rhs=mW3[:], start=True,
                             stop=True)
            nc.vector.tensor_copy(U8s[:], trP[:])
            for b in range(BC):
                for c in range(C):
                    nc.tensor.matmul(
                        lastTp[:, b:b + 1],
                        lhsT=NEall[:, (b * 8 + c) * 128:(b * 8 + c + 1) * 128],
                        rhs=U8s[:, b * 8 + c: b * 8 + c + 1],
                        start=(c == 0), stop=(c == C - 1))
            nc.vector.tensor_tensor(out=dlT[:], in0=lastTp, in1=lastTprev[:],
                                    op=ALU.subtract)
            nc.vector.tensor_copy(lastTprev[:], lastTp)
            nc.tensor.matmul(dqTp, lhsT=Wq2_s[:], rhs=dlT[:], start=True, stop=True)
            nc.vector.tensor_tensor(
                out=Qd[:].rearrange("p (b h) -> p b h", h=H),
                in0=dqTp.rearrange("p (b o) -> p b o", o=1).to_broadcast([128, BC, H]),
                in1=ohBIG_s[:].rearrange("p (b h) -> p b h", h=H),
                op=ALU.mult)

            # scores += K . dq
            for b in range(BC):
                for c in range(C):
                    nc.tensor.matmul(
                        scoresP[:, c * 128 + b * 8: c * 128 + b * 8 + 8],
                        lhsT=kTall[:, (b * 8 + c) * 128:(b * 8 + c + 1) * 128],
                        rhs=Qd[:, b * 8:(b + 1) * 8], start=False, stop=True)

            # softmax weights
            nc.scalar.activation(Ptil[:], scoresP[:], AF.Exp)

            # denominators Z[b,h] then 1/Z broadcast
            for c in range(C):
                nc.tensor.matmul(Zp, lhsT=onesCb[:],
                                 rhs=Ptil[:, c * 128:(c + 1) * 128],
                                 start=(c == 0), stop=(c == C - 1))
            nc.vector.reciprocal(rZf[:], Zp)
            nc.vector.tensor_copy(rZ[:], rZf[:])
            nc.tensor.matmul(rZbp, lhsT=onesRb[:], rhs=rZ[:], start=True, stop=True)

            # ctx per head
            for b in range(BC):
                for c in range(C):
                    nc.tensor.matmul(
                        ctxHp[:, b * 8:(b + 1) * 8],
                        lhsT=Vall[:, (b * 8 + c) * 128:(b * 8 + c + 1) * 128],
                        rhs=Ptil[:, c * 128 + b * 8: c * 128 + b * 8 + 8],
                        start=(c == 0), stop=(c == C - 1))

            # extract ctx^T [e,b] with 1/Z and head one-hot
            nc.scalar.copy(cxs[:], ctxHp)
            nc.vector.tensor_tensor(out=t5[:], in0=cxs[:], in1=rZbp, op=ALU.mult)
            nc.vector.tensor_tensor(out=t6[:], in0=t5[:], in1=ohBIG_s[:], op=ALU.mult)
            nc.vector.tensor_reduce(out=ctxT[:],
                                    in_=t6[:].rearrange("p (b h) -> p b h", h=H),
                                    axis=mybir.AxisListType.X, op=ALU.add)
            nc.vector.tensor_tensor(out=dcT[:], in0=ctxT[:], in1=ctxTprev[:],
                                    op=ALU.subtract)
            nc.vector.tensor_copy(ctxTprev[:], ctxT[:])

            # x delta, logits accumulation
            nc.tensor.matmul(dxTp, lhsT=Wml_b[:], rhs=dcT[:], start=True, stop=True)
            nc.vector.tensor_copy(dxT[:], dxTp)
            for b in range(BC):
                for c in range(C):
                    nc.tensor.matmul(
                        logitsP[:, b * 8 + c: b * 8 + c + 1],
                        lhsT=lkTall[:, (b * 8 + c) * 128:(b * 8 + c + 1) * 128],
                        rhs=dxT[:, b:b + 1], start=False, stop=True)

            # tanh(z) = 1 - 2/(exp(2z)+1)  (stays in the exp/ln table set)
            nc.scalar.activation(e2[:], logitsP, AF.Exp, scale=2.0)
            nc.vector.tensor_scalar(out=ep1[:], in0=e2[:], scalar1=1.0,
                                    scalar2=None, op0=ALU.add)
            nc.vector.reciprocal(rp[:], ep1[:])
            nc.vector.tensor_scalar(out=th[:], in0=rp[:], scalar1=-2.0,
                                    scalar2=1.0, op0=ALU.mult, op1=ALU.add)
            nc.vector.tensor_tensor(out=lm[:], in0=th[:], in1=maskBIGp, op=ALU.add)

            # argmax: pack value+chunk, reduce over chunks, transpose, max
            nc.vector.tensor_scalar(out=pk1[:], in0=lm[:], scalar1=PACKS,
                                    scalar2=RBIG, op0=ALU.mult, op1=ALU.add)
            nc.vector.tensor_scalar(out=pk1[:], in0=pk1[:], scalar1=RBIG,
                                    scalar2=8.0, op0=ALU.subtract, op1=ALU.mult)
            nc.vector.tensor_tensor(out=packed[:], in0=pk1[:], in1=CIDX_s[:],
                                    op=ALU.add)
            nc.vector.tensor_reduce(out=pc[:],
                                    in_=packed[:].rearrange("p (b c) -> p b c", c=C),
                                    axis=mybir.AxisListType.X, op=ALU.max)
            nc.tensor.transpose(trP[:BC, :], pc[:], ident_s[:])
            nc.vector.tensor_copy(pcTs[:], trP[:BC, :])
            nc.vector.max(out=mx[:], in_=pcTs[:])
            nc.vector.max_index(out=mi[:], in_max=mx[:], in_values=pcTs[:])

            # lse path
            nc.scalar.activation(w2[:], lm[:], AF.Exp, scale=TANH_CLIP)
            nc.tensor.matmul(Z2p, lhsT=onesCb[:], rhs=w2[:], start=True, stop=True)
            nc.vector.tensor_reduce(out=sumw[:],
                                    in_=Z2p.rearrange("o (b c) -> o b c", c=C),
                                    axis=mybir.AxisListType.X, op=ALU.add)
            nc.scalar.activation(lse1[:], sumw[:], LN_FUNC)
            nc.tensor.transpose(lseTp, lse1[:], ident_s[:1, :1])
            nc.vector.tensor_copy(lseTs[:], lseTp)

            # decode packed max -> qv (value), c*, p*, n*
            nc.vector.tensor_copy(pf[:], mi[:, 0:1])
            nc.vector.tensor_scalar(out=d1[:], in0=mx[:, 0:1], scalar1=0.125,
                                    scalar2=0.4375, op0=ALU.mult,
                                    op1=ALU.subtract)
            nc.vector.tensor_scalar(out=qv[:], in0=d1[:], scalar1=RBIG,
                                    scalar2=RBIG, op0=ALU.add, op1=ALU.subtract)
            nc.vector.tensor_scalar(out=tt1[:], in0=qv[:], scalar1=8.0,
                                    scalar2=None, op0=ALU.mult)
            nc.vector.tensor_tensor(out=cst[:], in0=mx[:, 0:1], in1=tt1[:],
                                    op=ALU.subtract)
            nc.vector.tensor_scalar(out=tt2[:], in0=cst[:], scalar1=128.0,
                                    scalar2=None, op0=ALU.mult)
            nc.vector.tensor_tensor(out=nst[:], in0=tt2[:], in1=pf[:], op=ALU.add)
            nc.vector.tensor_scalar(out=mv10[:], in0=qv[:],
                                    scalar1=TANH_CLIP / PACKS, scalar2=None,
                                    op0=ALU.mult)
            nc.vector.tensor_tensor(out=lp[:], in0=mv10[:], in1=lseTs[:],
                                    op=ALU.subtract)
            nc.vector.tensor_tensor(out=total[:], in0=total[:], in1=lp[:],
                                    op=ALU.add)
            # rebuild mask one-hots for the newly chosen node
            nc.vector.tensor_scalar(out=mU[:], in0=iotaP_s[:], scalar1=pf[:, 0:1],
                                    scalar2=MASK_ATT, op0=ALU.is_equal,
                                    op1=ALU.mult)
            nc.vector.tensor_scalar(out=mU9[:], in0=iotaP_s[:], scalar1=pf[:, 0:1],
                                    scalar2=MASK_LOG, op0=ALU.is_equal,
                                    op1=ALU.mult)
            nc.vector.tensor_scalar(out=mU1[:], in0=iotaP_s[:], scalar1=pf[:, 0:1],
                                    scalar2=None, op0=ALU.is_equal)
            nc.vector.tensor_scalar(out=mW2[:], in0=CBIDX_s[:], scalar1=cst[:, 0:1],
                                    scalar2=None, op0=ALU.is_equal)
            nc.vector.tensor_scalar(out=mW3[:], in0=CBIDX2_s[:], scalar1=cst[:, 0:1],
                                    scalar2=None, op0=ALU.is_equal)

        if steps > 0:
            if steps <= STATIC_MAX:
                for _ in range(steps):
                    body()
            else:
                with tc.For_i(0, steps):
                    body()

        nc.sync.dma_start(out_total[:], total[:])

        if debug:
            nc.sync.dma_start(dbg["d_lm"][:], lm[:])
            nc.sync.dma_start(dbg["d_th"][:], th[:])
            nc.sync.dma_start(dbg["d_mx"][:], mx[:])
            nc.sync.dma_start(dbg["d_mi"][:], mi[:])
            nc.sync.dma_start(dbg["d_nst"][:], nst[:])
            nc.sync.dma_start(dbg["d_ctxT"][:], ctxT[:])
            nc.sync.dma_start(dbg["d_sumw"][:], sumw[:])
            nc.sync.dma_start(dbg["d_lse"][:], lseTs[:])
            dsc = sb.tile([128, 512], f32, tag="dsc")
            for (nm, src) in (("d_ptil", Ptil[:]), ("d_kT", kTall[:, :1024]),
                              ("d_v", Vall[:, :1024]), ("d_scores", scoresP[:]),
                              ("d_logits", logitsP)):
                w = src.shape[-1]
                for o in range(0, w, 512):
                    ww = min(512, w - o)
                    nc.vector.tensor_copy(dsc[:, :ww], src[:, o:o + ww])
                    nc.sync.dma_start(dbg[nm][:, o:o + ww], dsc[:, :ww])

    return nc


# ------------------------------------------------------------ host prep
def make_inputs(node_embeddings, graph_embedding, Wqkv, bqkv, Wfix, bfix,
                Wstep, bstep, Wmlp, bmlp):
    """Build the per-core in_maps (host: only O(MB) slicing + tiny solves)."""
    f = np.float32
    ne = np.ascontiguousarray(node_embeddings, dtype=f)        # [B,N,E]
    ge = np.asarray(graph_embedding, f)
    Wqkv = np.asarray(Wqkv, f); bqkv = np.asarray(bqkv, f)
    Wfix = np.asarray(Wfix, f); bfix = np.asarray(bfix, f)
    Wstep = np.asarray(Wstep, f); bstep = np.asarray(bstep, f)
    Wmlp = np.asarray(Wmlp, f); bmlp = np.asarray(bmlp, f)

    sc = f(SCALE)
    Wk = Wqkv[:, 0:E]; Wv = Wqkv[:, E:2 * E]; Wlk = Wqkv[:, 2 * E:3 * E]
    bk = bqkv[0:E]; bv = bqkv[E:2 * E]; blk = bqkv[2 * E:3 * E]
    Wml = (sc * Wmlp)
    ctx0 = np.linalg.solve(Wmlp.T.astype(np.float64),
                           -bmlp.astype(np.float64)).astype(f)  # c* s.t. c*@Wmlp=-bmlp

    bvec = np.arange(BC, dtype=np.int64)
    consts = {
        "Wk": Wk, "Wv": Wv, "Wlk": Wlk,
        "bkT": bk.reshape(E, 1), "blkT": blk.reshape(E, 1),
        "bvR": bv.reshape(1, E),
        "Wfx": sc * Wfix, "W1": sc * Wstep[0:E], "Wq2": sc * Wstep[E:2 * E],
        "bq1R": (sc * (bfix + bstep)).reshape(1, E),
        "Wml": Wml,
        "ident": np.eye(128, dtype=f),
    }
    e_idx = np.arange(128)
    bh_b, bh_h = np.divmod(np.arange(128), 8)
    consts["ohBIG"] = ((e_idx[:, None] // 16) == bh_h[None, :]).astype(f)
    bc_b, bc_c = np.divmod(np.arange(128), 8)
    consts["CIDX"] = np.broadcast_to(bc_c[None, :], (128, 128)).astype(f)
    # CBIDX [b, (c,b',h)]: c if b'==b else -1
    cb_c = np.arange(NP) // 128
    cb_b = (np.arange(NP) % 128) // 8
    consts["CBIDX"] = np.where(cb_b[None, :] == bvec[:, None],
                               cb_c[None, :], -1).astype(f)
    consts["CBIDX2"] = np.where(bc_b[None, :] == bvec[:, None],
                                bc_c[None, :], -1).astype(f)
    consts["iotaP"] = np.broadcast_to(np.arange(128, dtype=f)[None, :],
                                      (BC, 128)).copy()
    consts["padU"] = (e_idx >= 104).astype(f).reshape(1, 128)
    pw2 = np.zeros((1, NP), f); pw2[0, 7 * 128:] = MASK_ATT
    consts["padW2"] = pw2
    pw3 = np.zeros((1, 128), f); pw3[0, bc_c == 7] = MASK_LOG
    consts["padW3"] = pw3
    u0 = np.zeros((BC, 128), f); u0[:, 0] = MASK_ATT
    consts["U0"] = u0
    u09 = np.zeros((BC, 128), f); u09[:, 0] = MASK_LOG
    consts["U09"] = u09
    u01 = np.zeros((BC, 128), f); u01[:, 0] = 1.0
    consts["U01"] = u01
    w20 = np.zeros((BC, NP), f)
    for b in range(BC):
        w20[b, 0 * 128 + b * 8: 0 * 128 + b * 8 + 8] = 1.0  # node 0 -> c=0,p=0
    consts["W20"] = w20
    w30 = np.zeros((BC, 128), f)
    w30[bvec, bvec * 8 + 0] = 1.0
    consts["W30"] = w30

    in_maps = []
    for core in range(NCORES):
        bs = slice(core * BC, (core + 1) * BC)
        m = dict(consts)
        m["ne"] = ne[bs].reshape(BC * N, E)
        m["geT"] = np.ascontiguousarray(ge[bs].T)
        m["ctxp0"] = np.broadcast_to(ctx0[:, None], (E, BC)).copy()
        in_maps.append(m)
    return in_maps


_cached = {}


def _get_nc(steps=STEPS, debug=False):
    key = (steps, debug)
    if key not in _cached:
        _cached[key] = build_kernel(steps, debug)
    return _cached[key]


def kernel(node_embeddings, graph_embedding, Wqkv, bqkv, Wfix, bfix,
           Wstep, bstep, Wmlp, bmlp, _steps=STEPS, _debug=False, _trace=False):
    nc = _get_nc(_steps, _debug)
    in_maps = make_inputs(node_embeddings, graph_embedding, Wqkv, bqkv,
                          Wfix, bfix, Wstep, bstep, Wmlp, bmlp)
    res = run_bass_kernel_spmd(nc, in_maps, core_ids=list(range(NCORES)),
                               trace=_trace)
    total = np.concatenate([res.results[i]["total"][:, 0]
                            for i in range(NCORES)])
    if _debug or _trace:
        return total.astype(np.float32), res
    return total.astype(np.float32)
